# revision 59
# baseline (speedup 1.0000x reference)
"""Trainium2 Bass kernel for nn_Block_62354335203350 (pre-LN transformer block).

Sharding (8 cores): batch (B=2) x 4-way tensor-parallel heads for attention;
ReduceScatter after the output projection moves to row-parallel FFN (full
W1/W2 resident per core, own 512 rows), so only ONE collective stage exists.

v2 structure:
- LN1 deferred: transpose RAW x, bn_stats on natural tile, fold the
  (x-mu)*rstd affine into QKV matmuls (PE rank-1 accumulates + one
  broadcast-row multiply). gamma/beta fold into weights host-side.
- V projection in DoubleRow fp8 (half cost), bf16 QT/KT/V65/est.
- Attention software-pipelined (scores one group ahead of AV); head output
  written directly into persistent hoT.
- Projection in fp8 DoubleRow (hoT fp8, Wp fp8 x32).
- FFN split 3+1: rows 0-383 run while the last ReduceScatter is in flight;
  W1/W2 SBUF-resident, prefetched during attention. bp folded into xo on
  host, b2 added via PE rank-1, beta2/gamma2 folded into W1/b1.
"""
import numpy as np
from contextlib import ExitStack

import concourse.bass as bass
import concourse.tile as tile
import concourse.mybir as mybir
from concourse import bacc, bass_utils

F32 = mybir.dt.float32
F32R = mybir.dt.float32r
BF16 = mybir.dt.bfloat16
F8 = mybir.dt.float8e4
DR = mybir.MatmulPerfMode.DoubleRow
AF = mybir.ActivationFunctionType
OP = mybir.AluOpType

B, T, E, H, HS = 2, 2048, 1024, 16, 64
FF = 4 * E
EPS = 1e-5
N_CORES = 8
H4 = H // 4          # 4 heads per core
EC = E // 128        # 8 E-chunks
FC = FF // 128       # 32 hidden chunks
RGROUPS = [[0, 1, 2, 3], [4, 5, 6, 7]]
PROJ_FP8 = True      # fp8 DoubleRow output projection


def _bcast_ap(handle, parts, n):
    return bass.AP(tensor=handle, offset=0, ap=[[0, parts], [1, n]])


def _pmajor_ap(handle, nblk):
    return bass.AP(tensor=handle, offset=0, ap=[[1, 128], [128, nblk]])


def build(has_beta1):
    nc = bacc.Bacc("TRN2", target_bir_lowering=False, num_devices=N_CORES)

    x = nc.declare_dram_parameter("x", [T, E], F32, isOutput=False)
    xo = nc.declare_dram_parameter("xo", [512, E], F32, isOutput=False)
    wq = nc.declare_dram_parameter("wq", [128, 2, EC, 128], F8, isOutput=False)
    wk = nc.declare_dram_parameter("wk", [128, 2, EC, 128], F8, isOutput=False)
    wv = nc.declare_dram_parameter("wv", [128, EC, H4 * HS], F8, isOutput=False)
    if PROJ_FP8:
        wp = nc.declare_dram_parameter("wp", [128, 2, E], F8, isOutput=False)
    else:
        wp = nc.declare_dram_parameter("wp", [128, 2, E], F32R, isOutput=False)
    w1 = nc.declare_dram_parameter("w1", [128, FC, EC, 128], F8, isOutput=False)
    w2 = nc.declare_dram_parameter("w2", [128, EC, FC, 128], F8, isOutput=False)
    b1 = nc.declare_dram_parameter("b1", [FF], F32, isOutput=False)
    b2r = nc.declare_dram_parameter("b2r", [1, E], F32R, isOutput=False)
    cqn = nc.declare_dram_parameter("cqn", [1, 2, 128], F32R, isOutput=False)
    ckn = nc.declare_dram_parameter("ckn", [1, 2, 128], F32R, isOutput=False)
    cvn = nc.declare_dram_parameter("cvn", [1, H4 * HS], F32R, isOutput=False)
    onesr = nc.declare_dram_parameter("onesr", [1, 512], F32R, isOutput=False)
    vones = nc.declare_dram_parameter("vones", [16, 64], BF16, isOutput=False)
    if has_beta1:
        bqr = nc.declare_dram_parameter("bqr", [1, 2, 128], F32R, isOutput=False)
        bkr = nc.declare_dram_parameter("bkr", [1, 2, 128], F32R, isOutput=False)
        bvr = nc.declare_dram_parameter("bvr", [1, H4 * HS], F32R, isOutput=False)
    out = nc.declare_dram_parameter("out", [512, E], F32, isOutput=True)

    with tile.TileContext(nc) as tc, ExitStack() as top:
        consts = top.enter_context(tc.tile_pool(name="consts", bufs=1))
        dram = top.enter_context(tc.tile_pool(name="dram", bufs=1, space="DRAM"))
        ffnw = top.enter_context(tc.tile_pool(name="ffnw", bufs=1))

        # ---------------- constants ----------------
        ident = consts.tile([128, 128], F32)
        nc.gpsimd.memset(ident, 0.0)
        nc.gpsimd.affine_select(out=ident, in_=ident, compare_op=OP.not_equal,
                                fill=1.0, base=0, pattern=[[-1, 128]],
                                channel_multiplier=1)
        identb = consts.tile([128, 128], BF16)
        nc.gpsimd.tensor_copy(identb, ident)
        # tri[p, f] = 1 if f >= p else 0 (keep mask for scores^T diag blocks)
        tri = consts.tile([128, 128], BF16)
        nc.gpsimd.memset(tri, 1.0)
        nc.gpsimd.affine_select(out=tri, in_=tri, compare_op=OP.is_ge,
                                fill=0.0, base=0, pattern=[[1, 128]],
                                channel_multiplier=-1)
        ones_row = consts.tile([1, 512], F32R)
        nc.sync.dma_start(out=ones_row, in_=onesr.ap()[0:1, :])
        zero_col = consts.tile([128, 1], F32)
        nc.gpsimd.memset(zero_col, 0.0)
        neg1_col = consts.tile([128, 1], F32)
        nc.gpsimd.memset(neg1_col, -1.0)
        inv16_col = consts.tile([128, 1], F32)
        nc.gpsimd.memset(inv16_col, 1.0 / 16.0)
        eps_col = consts.tile([128, 1], F32)
        nc.gpsimd.memset(eps_col, EPS)
        epsk_col = consts.tile([128, 1], F32)
        nc.gpsimd.memset(epsk_col, EPS * 1024.0)
        mln32_col = consts.tile([128, 1], F32)
        nc.gpsimd.memset(mln32_col, -3.4657359027997265)
        b1_sb = consts.tile([128, FC], F32)
        with tc.tile_wait_until(0.05):
            nc.sync.dma_start(out=b1_sb, in_=_pmajor_ap(b1, FC))
        b2_sb = consts.tile([1, E], F32R)
        with tc.tile_wait_until(0.006):
            nc.sync.dma_start(out=b2_sb, in_=b2r.ap()[0:1, :])
        cq_sb = consts.tile([1, 2, 128], F32R)
        nc.sync.dma_start(out=cq_sb, in_=cqn.ap())
        ck_sb = consts.tile([1, 2, 128], F32R)
        nc.sync.dma_start(out=ck_sb, in_=ckn.ap())
        cv_sb = consts.tile([1, H4 * HS], F32R)
        nc.sync.dma_start(out=cv_sb, in_=cvn.ap()[0:1, :])
        if has_beta1:
            bq_sb = consts.tile([1, 2, 128], F32R)
            nc.sync.dma_start(out=bq_sb, in_=bqr.ap())
            bk_sb = consts.tile([1, 2, 128], F32R)
            nc.sync.dma_start(out=bk_sb, in_=bkr.ap())
            bv_sb = consts.tile([1, H4 * HS], F32R)
            nc.sync.dma_start(out=bv_sb, in_=bvr.ap()[0:1, :])

        # DRAM bounces for the ReduceScatters
        rs_in = dram.tile([T, E], F8)
        rsos = [dram.tile([128, E], F8, name=f"rso{i}") for i in range(4)]

        # FFN weights: resident all kernel, loaded during attention
        w1_sb = ffnw.tile([128, FC, EC, 128], F8)
        w2_sb = ffnw.tile([128, EC, FC, 128], F8)

        # ---------------- LN2-front pool (spans phases 2+3) ----------------
        ln2p = top.enter_context(tc.tile_pool(name="ln2p", bufs=4))
        ffp = top.enter_context(tc.tile_pool(name="ffp", bufs=1))
        x2 = ffp.tile([128, 4, E], F32)
        h2ts = [None] * 4

        def ln2_front(a):
            """residual + LN2 stats + normalized h2_t (no PSUM use)."""
            rso_sb = ln2p.tile([128, E], F8, tag="rso_sb", name=f"rso_sb{a}", bufs=2)
            nc.sync.dma_start(out=rso_sb, in_=rsos[a][:, :])
            xoa = ln2p.tile([128, E], F32, tag="xoa", name=f"xoa{a}", bufs=2)
            with tc.tile_wait_until(0.075 + 0.02 * a):
                nc.sync.dma_start(out=xoa, in_=xo.ap()[a * 128:(a + 1) * 128, :])
            nc.vector.scalar_tensor_tensor(
                out=x2[:, a, :], in0=rso_sb, scalar=inv16_col, in1=xoa,
                op0=OP.mult, op1=OP.add)
            xg2 = x2[:, a, :].rearrange("p (sg f) -> p sg f", f=512)
            stats = ln2p.tile([128, 2, 6], F32, tag="st6b", name=f"st6b{a}", bufs=2)
            for sg in range(2):
                nc.vector.bn_stats(out=stats[:, sg, :], in_=xg2[:, sg, :])
            mv = ln2p.tile([128, 2], F32, tag="mvb", name=f"mvb{a}")
            nc.vector.bn_aggr(out=mv, in_=stats)
            std = ln2p.tile([128, 1], F32, tag="stdb", name=f"stdb{a}")
            nc.scalar.activation(out=std, in_=mv[:, 1:2], func=AF.Sqrt,
                                 bias=eps_col)
            rstd = ln2p.tile([128, 1], F32, tag="rstdb", name=f"rstdb{a}")
            nc.vector.reciprocal(out=rstd, in_=std)
            nmr = ln2p.tile([128, 1], F32, tag="nmrb", name=f"nmrb{a}")
            nc.vector.tensor_scalar(out=nmr, in0=mv[:, 0:1],
                                    scalar1=rstd, scalar2=neg1_col,
                                    op0=OP.mult, op1=OP.mult)
            h2_t = ln2p.tile([128, E], BF16, tag="h2_t", name=f"h2_t{a}")
            nc.scalar.activation(out=h2_t, in_=x2[:, a, :], func=AF.Identity,
                                 scale=rstd, bias=nmr)
            h2ts[a] = h2_t

        # ---------------- attention-persistent tiles ----------------
        attn_scope = ExitStack()
        persist = attn_scope.enter_context(tc.tile_pool(name="attn_persist", bufs=1))
        QT = persist.tile([128, 2, T], BF16)       # [2x64 dims, pair, qrow]
        KT = persist.tile([128, 2, T], BF16)
        V65 = persist.tile([128, 16, H4, 65], BF16)  # [row%128, rowtile, head, hs+1]
        with tc.tile_wait_until(0.010):
            nc.sync.dma_start(
                out=V65[:, :, :, 64],
                in_=bass.AP(tensor=vones, offset=0, ap=[[0, 128], [4, 16], [1, 4]]))
        hoT = persist.tile([128, 2, T], F8 if PROJ_FP8 else BF16)

        # ---------------- phase 1: x^T + stats + QKV (deferred LN) -------
        with ExitStack() as ph:
            qkvw = ph.enter_context(tc.tile_pool(name="qkvw", bufs=1))
            lnp = ph.enter_context(tc.tile_pool(name="lnp", bufs=4))
            htsp = ph.enter_context(tc.tile_pool(name="htsp", bufs=3))
            rowp = ph.enter_context(tc.tile_pool(name="rowp", bufs=2))
            pst = ph.enter_context(tc.tile_pool(name="pst", bufs=2, space="PSUM"))
            psq = ph.enter_context(tc.tile_pool(name="psq", bufs=2, space="PSUM"))
            psr = ph.enter_context(tc.tile_pool(name="psr", bufs=1, space="PSUM"))

            wq_sb = qkvw.tile([128, 2, EC, 128], F8)
            nc.scalar.dma_start(out=wq_sb, in_=wq.ap())
            wk_sb = qkvw.tile([128, 2, EC, 128], F8)
            nc.scalar.dma_start(out=wk_sb, in_=wk.ap())
            wv_sb = qkvw.tile([128, EC, H4 * HS], F8)
            nc.scalar.dma_start(out=wv_sb, in_=wv.ap())

            copy_eng = [nc.scalar, nc.vector, nc.scalar, nc.gpsimd]
            for s in range(4):  # row slices of 512
                hts = htsp.tile([128, EC, 512], F8, tag="hts")
                # per-slice stat rows (partition 0): mu[q], rstd[q]/32
                srow_mu = rowp.tile([1, 512], F32R, tag="srow_mu", bufs=1)
                srow_rs = rowp.tile([1, 512], F32R, tag="srow_rs", bufs=1)
                rstds = []
                for rt in range(4):
                    row0 = s * 512 + rt * 128
                    x_t = lnp.tile([128, E], F32, tag="x_t", bufs=3)
                    nc.sync.dma_start(out=x_t, in_=x.ap()[row0:row0 + 128, :])
                    # stats on natural tile
                    xg = x_t.rearrange("p (sg f) -> p sg f", f=512)
                    stats = lnp.tile([128, 2, 6], F32, tag="st6")
                    for sg in range(2):
                        nc.vector.bn_stats(out=stats[:, sg, :], in_=xg[:, sg, :])
                    mv = lnp.tile([128, 2], F32, tag="mv")
                    nc.vector.bn_aggr(out=mv, in_=stats)
                    std32 = lnp.tile([128, 1], F32, tag="std32")
                    nc.scalar.activation(out=std32, in_=mv[:, 1:2], func=AF.Sqrt,
                                         bias=epsk_col, scale=1024.0)
                    rstd32 = lnp.tile([128, 1], F32, tag="rstd32")
                    nc.vector.reciprocal(out=rstd32, in_=std32)
                    rstds.append(rstd32)
                    # transpose stats cols -> rows on partition 0
                    pstk = psr.tile([1, 256], F32, tag="pstk")
                    nc.tensor.transpose(pstk[:, 0:128], mv[:, 0:1], ident)
                    nc.tensor.transpose(pstk[:, 128:256], rstd32, ident)
                    nc.scalar.copy(out=srow_mu[:, rt * 128:(rt + 1) * 128],
                                   in_=pstk[:, 0:128])
                    nc.scalar.copy(out=srow_rs[:, rt * 128:(rt + 1) * 128],
                                   in_=pstk[:, 128:256])
                    # transpose raw x -> hts (fp8)
                    pt = pst.tile([128, EC, 128], F32, tag="pt")
                    for c in range(EC):
                        nc.tensor.transpose(pt[:, c, :], x_t[:, c * 128:(c + 1) * 128], ident)
                    ce = copy_eng[rt]
                    if ce is nc.vector:
                        nc.vector.tensor_copy(hts[:, :, rt * 128:(rt + 1) * 128], pt)
                    elif ce is nc.gpsimd:
                        nc.gpsimd.tensor_copy(hts[:, :, rt * 128:(rt + 1) * 128], pt)
                    else:
                        nc.scalar.copy(out=hts[:, :, rt * 128:(rt + 1) * 128], in_=pt)
                # rstd/32 broadcast [128, 512] for this slice
                psb = psr.tile([128, 512], F32, tag="psb")
                nc.tensor.matmul(psb, ones_row[0:1, 0:128], srow_rs,
                                 start=True, stop=True)
                bc = rowp.tile([128, 512], BF16, tag="bc")
                nc.scalar.copy(out=bc, in_=psb)
                # QT/KT for this slice
                for di, (dst, wsb, csb, bsb) in enumerate((
                        (QT, wq_sb, cq_sb, "bq"), (KT, wk_sb, ck_sb, "bk"))):
                    for p in range(2):
                        ps = psq.tile([128, 512], F32, tag="ps_qk")
                        for c in range(EC // 2):
                            nc.tensor.matmul(ps, wsb[:, p, 2 * c:2 * c + 2, :],
                                             hts[:, 2 * c:2 * c + 2, :],
                                             start=(c == 0), stop=False,
                                             perf_mode=DR)
                        if has_beta1:
                            bb = bq_sb if bsb == "bq" else bk_sb
                            nc.tensor.matmul(ps, bb[0:1, p, :], ones_row,
                                             start=False, stop=False)
                        nc.tensor.matmul(ps, csb[0:1, p, :], srow_mu,
                                         start=False, stop=True)
                        if (di + p) % 2 == 0:
                            nc.vector.tensor_mul(
                                dst[:, p, s * 512:(s + 1) * 512], ps, bc)
                        else:
                            nc.gpsimd.tensor_mul(
                                dst[:, p, s * 512:(s + 1) * 512], ps, bc)
                # V for this slice (natural rows on partitions, DoubleRow)
                for rt in range(4):
                    psv = psq.tile([128, 512], F32, tag="ps_qk")
                    for c in range(EC // 2):
                        nc.tensor.matmul(psv[:, 0:H4 * HS],
                                         hts[:, 2 * c:2 * c + 2, rt * 128:(rt + 1) * 128],
                                         wv_sb[:, 2 * c:2 * c + 2, :],
                                         start=(c == 0), stop=False,
                                         perf_mode=DR)
                    if has_beta1:
                        nc.tensor.matmul(psv[:, 0:H4 * HS], ones_row[0:1, 0:128],
                                         bv_sb, start=False, stop=False)
                    nc.tensor.matmul(psv[:, 0:H4 * HS],
                                     srow_mu[0:1, rt * 128:(rt + 1) * 128],
                                     cv_sb, start=False, stop=True)
                    vout = V65[:, s * 4 + rt, :, 0:64]
                    pv3 = psv.rearrange("p (g d) -> p g d", d=64)[:, 0:H4, :]
                    if rt % 2 == 0:
                        nc.vector.tensor_scalar(
                            out=vout, in0=pv3,
                            scalar1=rstds[rt], scalar2=zero_col,
                            op0=OP.mult, op1=OP.add)
                    else:
                        nc.gpsimd.tensor_scalar(
                            out=vout, in0=pv3,
                            scalar1=rstds[rt], scalar2=zero_col,
                            op0=OP.mult, op1=OP.add)

        # ---------------- phase 2: attention + projection + RS ------------
        with ExitStack() as ph:
            wpp = ph.enter_context(tc.tile_pool(name="wpp", bufs=1))
            estp = ph.enter_context(tc.tile_pool(name="estp", bufs=6))
            prp = ph.enter_context(tc.tile_pool(name="prp", bufs=4))
            psst = ph.enter_context(tc.tile_pool(name="psst", bufs=3, space="PSUM"))
            psav = ph.enter_context(tc.tile_pool(name="psav", bufs=2, space="PSUM"))

            wp_sb = wpp.tile([128, 2, E], F8 if PROJ_FP8 else F32R)
            with tc.tile_wait_until(0.022):
                nc.scalar.dma_start(out=wp_sb, in_=wp.ap())

            def emit_attn(qt, h):
                """scores/exp/mask/AV for one head, software-pipelined."""
                q0 = qt * 512
                p, off = h // 2, (h % 2) * 64
                nkb = 4 * qt + 4
                ng = nkb // 2
                av = psav.tile([65, 512], F32, tag="av", name=f"av{qt}_{h}")
                ests = {}

                def emit_st(g):
                    st = psst.tile([128, 2, 512], F32, tag="st",
                                   name=f"st{qt}_{h}_{g}")
                    est = estp.tile([128, 2, 512], BF16, tag="est",
                                    name=f"est{qt}_{h}_{g}")
                    for j2 in range(2):
                        kb = g * 2 + j2
                        dj = kb - 4 * qt
                        qoff = dj * 128 if dj >= 0 else 0
                        nc.tensor.matmul(
                            st[:, j2, qoff:512],
                            KT[off:off + 64, p, kb * 128:(kb + 1) * 128],
                            QT[off:off + 64, p, q0 + qoff:q0 + 512],
                            start=True, stop=True)
                    dj0 = g * 2 - 4 * qt
                    if dj0 >= 2:
                        for j2 in range(2):
                            qo = (dj0 + j2) * 128
                            nc.scalar.activation(out=est[:, j2, qo:512],
                                                 in_=st[:, j2, qo:512],
                                                 func=AF.Exp,
                                                 scale=float(HS) ** -0.5)
                    else:
                        nc.scalar.activation(out=est, in_=st, func=AF.Exp,
                                             scale=float(HS) ** -0.5)
                    for j2 in range(2):
                        kb = g * 2 + j2
                        dj = kb - 4 * qt
                        if dj >= 0:
                            qoff = dj * 128
                            nc.vector.tensor_mul(
                                est[:, j2, qoff:qoff + 128],
                                est[:, j2, qoff:qoff + 128], tri)
                    ests[g] = est

                def emit_av(g):
                    est = ests.pop(g)
                    for j2 in range(2):
                        kb = g * 2 + j2
                        dj = kb - 4 * qt
                        qoff = dj * 128 if dj >= 0 else 0
                        nc.tensor.matmul(
                            av[:, qoff:512],
                            V65[:, kb, h, :],
                            est[:, j2, qoff:512],
                            start=(kb == 0), stop=(kb == nkb - 1))

                emit_st(0)
                for g in range(1, ng):
                    emit_st(g)
                    emit_av(g - 1)
                emit_av(ng - 1)
                return av

            def emit_tail(qt, h, av):
                """softmax normalize + write into hoT."""
                q0 = qt * 512
                p, off = h // 2, (h % 2) * 64
                recip = estp.tile([1, 512], F32R, tag="recip",
                                  name=f"recip{qt}_{h}", bufs=3)
                with nc.allow_low_precision(reason="f32r is fp32-width"):
                    nc.vector.reciprocal(out=recip, in_=av[64:65, :])
                rb = psst.tile([64, 512], F32, tag="st", name=f"rb{qt}_{h}")
                nc.tensor.matmul(rb, ones_row[0:1, 0:64], recip,
                                 start=True, stop=True)
                rbs = estp.tile([64, 512], BF16, tag="rbs", name=f"rbs{qt}_{h}",
                                bufs=2)
                nc.vector.tensor_copy(rbs, rb)
                nc.vector.tensor_mul(hoT[off:off + 64, p, q0:q0 + 512],
                                     av[0:64, :], rbs)

            def emit_proj(qt):
                q0 = qt * 512
                for rb2 in range(4):
                    r0 = q0 + rb2 * 128
                    prt = prp.tile([128, E], F8, tag="prt", name=f"prt{qt}_{rb2}")
                    for eh in range(2):
                        pr = psst.tile([128, 512], F32, tag="st",
                                       name=f"pr{qt}_{rb2}_{eh}")
                        if PROJ_FP8:
                            nc.tensor.matmul(pr, hoT[:, 0:2, r0:r0 + 128],
                                             wp_sb[:, 0:2, eh * 512:(eh + 1) * 512],
                                             start=True, stop=True, perf_mode=DR)
                            wscale = 16.0 / 32.0
                        else:
                            for p in range(2):
                                nc.tensor.matmul(pr, hoT[:, p, r0:r0 + 128],
                                                 wp_sb[:, p, eh * 512:(eh + 1) * 512],
                                                 start=(p == 0), stop=(p == 1))
                            wscale = 16.0
                        if eh == 0:
                            nc.vector.tensor_scalar(
                                out=prt[:, 0:512], in0=pr,
                                scalar1=wscale, scalar2=None, op0=OP.mult)
                        else:
                            nc.gpsimd.tensor_scalar(
                                out=prt[:, 512:1024], in0=pr,
                                scalar1=wscale, scalar2=None, op0=OP.mult)
                    nc.gpsimd.dma_start(out=rs_in[r0:r0 + 128, :], in_=prt)
                nc.gpsimd.collective_compute(
                    "ReduceScatter", OP.add, replica_groups=RGROUPS,
                    ins=[rs_in[qt * 512:(qt + 1) * 512, :].opt()],
                    outs=[rsos[qt].opt()])

            # interleave map: block-a LN2 front placed at (qt, after-head) so
            # rso_a (ready ~18us after proj_a) arrives before DVE gets there
            fronts = {(2, 1): 0, (3, 0): 1, (3, 2): 2}
            for qt in range(4):
                avs = {}
                for h in range(H4):
                    avs[h] = emit_attn(qt, h)
                    if h >= 1:
                        emit_tail(qt, h - 1, avs.pop(h - 1))
                    if (qt, h) in fronts:
                        a = fronts[(qt, h)]
                        with tc.tile_wait_until(0.112 + 0.030 * a):
                            ln2_front(a)
                emit_tail(qt, H4 - 1, avs.pop(H4 - 1))
                emit_proj(qt)
                if qt == 0:
                    with tc.tile_wait_until(0.045):
                        for i in range(4):
                            nc.scalar.dma_start(out=w1_sb[:, 8 * i:8 * (i + 1)],
                                                in_=w1.ap()[:, 8 * i:8 * (i + 1)])
                if qt == 1:
                    with tc.tile_wait_until(0.065):
                        for i in range(4):
                            nc.scalar.dma_start(out=w2_sb[:, 2 * i:2 * (i + 1)],
                                                in_=w2.ap()[:, 2 * i:2 * (i + 1)])
        attn_scope.close()

        # ---------------- phase 3: FFN (split 3+1) ----------------
        with ExitStack() as ph:
            outp = ph.enter_context(tc.tile_pool(name="outp", bufs=4))
            psf = ph.enter_context(tc.tile_pool(name="psf", bufs=3, space="PSUM"))
            pst2 = ph.enter_context(tc.tile_pool(name="pst2", bufs=2, space="PSUM"))

            o_ts = [None] * 4
            w_eng = [nc.scalar, nc.vector, nc.gpsimd]

            def transpose_block(a, h2T, ai):
                pt2 = pst2.tile([128, EC, 128], BF16, tag="pt2", name=f"pt2{a}")
                for c in range(EC):
                    nc.tensor.transpose(pt2[:, c, :],
                                        h2ts[a][:, c * 128:(c + 1) * 128], identb)
                if ai % 2 == 0:
                    nc.vector.tensor_copy(h2T[:, :, ai * 128:(ai + 1) * 128], pt2)
                else:
                    nc.gpsimd.tensor_copy(h2T[:, :, ai * 128:(ai + 1) * 128], pt2)

            def ffn1(h2T, ff1T, N):
                for m in range(FC):
                    ps1 = psf.tile([128, 512], F32, tag="psf", name=f"ps1_{N}_{m}")
                    for c in range(EC // 2):
                        nc.tensor.matmul(ps1[:, 0:N], w1_sb[:, m, 2 * c:2 * c + 2, :],
                                         h2T[:, 2 * c:2 * c + 2, :],
                                         start=(c == 0), stop=(c == EC // 2 - 1),
                                         perf_mode=DR)
                    we = w_eng[m % 3]
                    if we is nc.scalar:
                        nc.scalar.activation(out=ff1T[:, m, :], in_=ps1[:, 0:N],
                                             func=AF.Relu,
                                             bias=b1_sb[:, m:m + 1], scale=1.0)
                    else:
                        we.tensor_scalar(out=ff1T[:, m, :], in0=ps1[:, 0:N],
                                         scalar1=b1_sb[:, m:m + 1],
                                         scalar2=zero_col,
                                         op0=OP.add, op1=OP.max)

            def ffn2(ff1T, alist, N):
                for e in range(EC):
                    ps2 = psf.tile([128, 512], F32, tag="psf", name=f"ps2_{N}_{e}")
                    for c in range(FC // 2):
                        nc.tensor.matmul(ps2[:, 0:N], w2_sb[:, e, 2 * c:2 * c + 2, :],
                                         ff1T[:, 2 * c:2 * c + 2, :],
                                         start=(c == 0), stop=False,
                                         perf_mode=DR)
                    nc.tensor.matmul(ps2[:, 0:N], b2_sb[0:1, e * 128:(e + 1) * 128],
                                     ones_row[0:1, 0:N], start=False, stop=True)
                    f2s = ln2p.tile([128, 512], F32, tag="f2s", name=f"f2s_{N}_{e}", bufs=3)
                    we = w_eng[e % 3]
                    if we is nc.scalar:
                        nc.scalar.activation(out=f2s[:, 0:N], in_=ps2[:, 0:N],
                                             func=AF.Copy, scale=1.0 / 64.0)
                    else:
                        we.tensor_scalar(out=f2s[:, 0:N], in0=ps2[:, 0:N],
                                         scalar1=1.0 / 64.0, scalar2=None,
                                         op0=OP.mult)
                    tps = pst2.tile([128, 4, 128], F32, tag="tps", name=f"tps_{N}_{e}")
                    for ai, a in enumerate(alist):
                        nc.tensor.transpose(tps[:, ai, :],
                                            f2s[:, ai * 128:(ai + 1) * 128], ident)
                    for ai, a in enumerate(alist):
                        if (e + ai) % 2 == 0:
                            nc.vector.tensor_add(
                                o_ts[a][:, e * 128:(e + 1) * 128],
                                tps[:, ai, :], x2[:, a, e * 128:(e + 1) * 128])
                        else:
                            nc.gpsimd.tensor_add(
                                o_ts[a][:, e * 128:(e + 1) * 128],
                                tps[:, ai, :], x2[:, a, e * 128:(e + 1) * 128])

            # pass A: rows 0-383 (overlaps the in-flight RS3)
            h2T_A = ffp.tile([128, EC, 384], F8)
            for ai, a in enumerate([0, 1, 2]):
                transpose_block(a, h2T_A, ai)
            ff1T_A = ffp.tile([128, FC, 384], F8)
            for a in [0, 1, 2]:
                o_ts[a] = outp.tile([128, E], F32, tag="o_t", name=f"o_t{a}")
            o_ts[3] = outp.tile([128, E], F32, tag="o_t", name="o_t3")
            ffn1(h2T_A, ff1T_A, 384)
            with tc.tile_wait_until(0.210):
                ln2_front(3)
            ffn2(ff1T_A, [0, 1, 2], 384)
            for a in [0, 1, 2]:
                nc.sync.dma_start(out=out.ap()[a * 128:(a + 1) * 128, :],
                                  in_=o_ts[a])
            # pass B: rows 384-511
            with tc.tile_wait_until(0.2185):
                h2T_B = ffp.tile([128, EC, 128], F8)
                transpose_block(3, h2T_B, 0)
                ff1T_B = ffp.tile([128, FC, 128], F8)
                ffn1(h2T_B, ff1T_B, 128)
                ffn2(ff1T_B, [3], 128)
                nc.sync.dma_start(out=out.ap()[384:512, :], in_=o_ts[3])

    nc.compile()
    return nc


_CACHE = {}


def _get_nc(flags):
    if flags not in _CACHE:
        _CACHE[flags] = build(*flags)
    return _CACHE[flags]


def kernel(x, Wq, Wk, Wv, Wp, bp, W1, b1, W2, b2, g1, beta1, g2, beta2):
    x = np.asarray(x, np.float32)
    Wq, Wk, Wv = (np.asarray(a, np.float32) for a in (Wq, Wk, Wv))
    Wp, bp = np.asarray(Wp, np.float32), np.asarray(bp, np.float32)
    W1, b1 = np.asarray(W1, np.float32), np.asarray(b1, np.float32)
    W2, b2 = np.asarray(W2, np.float32), np.asarray(b2, np.float32)
    g1, beta1 = np.asarray(g1, np.float32), np.asarray(beta1, np.float32)
    g2, beta2 = np.asarray(g2, np.float32), np.asarray(beta2, np.float32)

    has_beta1 = bool(not np.all(beta1 == 0.0))
    nc = _get_nc((has_beta1,))

    F8NP = mybir.dt.np(F8)
    BFNP = mybir.dt.np(BF16)

    # fold gamma into weights (free host-side)
    Wq_e = Wq * g1[None, :, None]
    Wk_e = Wk * g1[None, :, None]
    Wv_e = Wv * g1[None, :, None]
    W1_e = W1 * g2[:, None]
    b1_host = (b1 + beta2 @ W1) * 32.0
    b2_host = (b2 * 64.0).reshape(1, E)

    # FFN weights, p-major resident layouts
    w1_blocks = np.ascontiguousarray(
        (W1_e * 32.0).reshape(EC, 128, FC, 128).transpose(1, 2, 0, 3)).astype(F8NP)
    w2_blocks = np.ascontiguousarray(
        (W2 * 2.0).reshape(FC, 128, EC, 128).transpose(1, 2, 0, 3)).astype(F8NP)

    def pmaj(w):  # [E, n] -> [128, E//128, n]
        ec = w.shape[0] // 128
        return np.ascontiguousarray(w.reshape(ec, 128, w.shape[1]).transpose(1, 0, 2))

    in_maps = []
    for c in range(N_CORES):
        b, r = divmod(c, 4)
        h0 = 4 * r
        own = [slice(512 * qt + 128 * r, 512 * qt + 128 * r + 128) for qt in range(4)]
        wq4 = Wq_e[h0:h0 + 4].transpose(1, 0, 2).reshape(E, 256) * 32.0
        wk4 = Wk_e[h0:h0 + 4].transpose(1, 0, 2).reshape(E, 256) * 32.0
        wv4 = Wv_e[h0:h0 + 4].transpose(1, 0, 2).reshape(E, 256) * 32.0
        wp4 = Wp[h0 * HS:(h0 + 4) * HS]
        m = {
            "x": np.ascontiguousarray(x[b]),
            "xo": np.ascontiguousarray(
                np.concatenate([x[b][sl] for sl in own], 0) + bp[None, :]),
            "wq": np.ascontiguousarray(
                pmaj(wq4).reshape(128, EC, 2, 128).transpose(0, 2, 1, 3)).astype(F8NP),
            "wk": np.ascontiguousarray(
                pmaj(wk4).reshape(128, EC, 2, 128).transpose(0, 2, 1, 3)).astype(F8NP),
            "wv": pmaj(wv4).astype(F8NP),
            "wp": (pmaj(wp4 * 32.0).astype(F8NP) if PROJ_FP8
                   else pmaj(wp4)),
            "w1": w1_blocks, "w2": w2_blocks,
            "b1": b1_host, "b2r": b2_host,
            "cqn": np.ascontiguousarray(
                (-wq4.astype(F8NP).astype(np.float32).sum(0)).reshape(1, 2, 128)),
            "ckn": np.ascontiguousarray(
                (-wk4.astype(F8NP).astype(np.float32).sum(0)).reshape(1, 2, 128)),
            "cvn": (-wv4.astype(F8NP).astype(np.float32).sum(0)).reshape(1, 256),
            "onesr": np.ones((1, 512), np.float32),
            "vones": np.ones((16, 64), BFNP),
        }
        if has_beta1:
            m["bqr"] = (beta1 @ wq4).reshape(1, 2, 128).astype(np.float32)
            m["bkr"] = (beta1 @ wk4).reshape(1, 2, 128).astype(np.float32)
            m["bvr"] = (beta1 @ wv4).reshape(1, 256).astype(np.float32)
        in_maps.append(m)

    res = bass_utils.run_bass_kernel_spmd(nc, in_maps, core_ids=list(range(N_CORES)))

    outp = np.empty((B, T, E), np.float32)
    for c in range(N_CORES):
        b, r = divmod(c, 4)
        o = res.results[c]["out"]
        for qt in range(4):
            outp[b, 512 * qt + 128 * r:512 * qt + 128 * r + 128] = \
                o[128 * qt:128 * qt + 128]
    return outp
